# revision 16
# baseline (speedup 1.0000x reference)
"""Trainium2 Bass kernel for 3-layer GraphSAGE (nn_DeviceGNN).

Restructured algebra (validated: exact in f32):
  feat_0 = emb'[degree]            emb' = [emb | 1]  (97 cols)
  aggU_0 = C @ emb'                C = (dst x srctype) histogram, via one-hot matmuls
  Z_l    = D aggU_l                D = diag(1/max(indeg,1))
  feat_{l+1} = feat_l @ Ws_l' + Z_l @ Wn_l'     (W' = 97x97 with bias row + ones col)
  M_l    = A @ Z_l                 SpMM: per-edge gather (dma_gather) + one-hot
                                   segsum matmuls; only l=0,1 needed
  aggU_{l+1} = aggU_l @ Ws_l' + M_l @ Wn_l'
  output = feat_3[:, :96]

Sharding: nodes/edges by destination across 8 cores (6272 dst rows each).
Z tables are AllGathered between SpMMs. Everything bf16 except PSUM (f32
accumulate) and the final output.
"""
import os
import sys

sys.path.insert(0, "/opt/trn_rl_repo")
import numpy as np
import ml_dtypes

bfloat16 = ml_dtypes.bfloat16

N = 50000
NP = 50176
D = 96
DP = 97
NTYPES = 64
NCORES = 8
SHARD = NP // NCORES  # 6272
GP = SHARD // 128  # 49 groups per core
BLK = 128


def _prep(degree, edge_src, edge_dst, emb, Wlist):
    """Host-side sharding/metadata prep. Returns (in_maps, BE, BO, offsets)."""
    deg = np.asarray(degree).astype(np.int64)
    es = np.asarray(edge_src).astype(np.int64)
    ed = np.asarray(edge_dst).astype(np.int64)

    order = np.argsort(ed, kind="stable")
    es_s = es[order]
    ed_s = ed[order]
    # group id of each (sorted) edge; groups are global: 0..391
    gid = ed_s // 128
    # boundaries of each global group in the sorted edge list
    bounds = np.searchsorted(gid, np.arange(NP // 128 + 1))

    # per (core, group-in-core): even/odd edge lists
    ecnt = np.zeros((NCORES, GP), np.int64)
    ocnt = np.zeros((NCORES, GP), np.int64)
    elists = [[None] * GP for _ in range(NCORES)]
    for c in range(NCORES):
        for g in range(GP):
            G = c * GP + g
            lo, hi = bounds[G], bounds[G + 1]
            s = es_s[lo:hi]
            dloc = ed_s[lo:hi] - G * 128  # 0..127
            even = s % 2 == 0
            elists[c][g] = (s[even], dloc[even], s[~even], dloc[~even])
            ecnt[c, g] = even.sum()
            ocnt[c, g] = hi - lo - ecnt[c, g]

    BE = np.maximum(1, -(-ecnt.max(axis=0) // 128))  # [GP] blocks, >=1
    BO = np.maximum(1, -(-ocnt.max(axis=0) // 128))
    EB = int(BE.sum())
    OB = int(BO.sum())
    CBLK = 16  # blocks per dma_gather call (2048 descriptors)
    EBp = -(-EB // CBLK) * CBLK
    OBp = -(-OB // CBLK) * CBLK
    NB = EBp + OBp  # total block-columns per core (E-major then O-major)
    NI = NB * 8

    ecol = np.zeros(GP, np.int64)
    ocol = np.zeros(GP, np.int64)
    acc = 0
    for g in range(GP):
        ecol[g] = acc
        acc += BE[g]
    acc = EBp
    for g in range(GP):
        ocol[g] = acc
        acc += BO[g]

    # (dst x srctype) histogram, host-side index preprocessing
    Ch = np.zeros((NP, NTYPES), np.float32)
    np.add.at(Ch, (ed, deg[es]), 1.0)

    in_maps = []
    for c in range(NCORES):
        idxcols = np.zeros((NB, 128), np.int64)  # [blockcol, slot] pair idx
        ldst = np.full((NB, 128), -1.0, np.float32)
        for g in range(GP):
            se, de, so, do = elists[c][g]
            for (sv, dl, nblk, boff) in (
                (se, de, int(BE[g]), int(ecol[g])),
                (so, do, int(BO[g]), int(ocol[g])),
            ):
                nslot = nblk * 128
                idx = np.zeros(nslot, np.int64)
                idx[: len(sv)] = sv >> 1
                ld = np.full(nslot, -1.0, np.float32)
                ld[: len(sv)] = dl
                idxcols[boff : boff + nblk, :] = idx.reshape(nblk, 128)
                ldst[boff : boff + nblk, :] = ld.reshape(nblk, 128)

        # wrap idx per call: call k covers block-cols [16k,16k+16); desc i of
        # the call -> [i%16, k*128 + i//16]
        idxw = np.zeros((128, NI), np.int16)
        for k in range(NB // CBLK):
            flat = idxcols[k * CBLK : (k + 1) * CBLK, :].reshape(-1)  # 2048
            w = flat.reshape(-1, 16).T.astype(np.int16)  # [16, 128]
            idxw[:, k * 128 : (k + 1) * 128] = np.tile(w, (8, 1))

        ldst = ldst.T  # [128 slots, NB]
        degp = np.zeros(SHARD, np.int64)
        own = deg[c * SHARD : min((c + 1) * SHARD, N)]
        degp[: len(own)] = own
        degb = np.tile(degp[None, :], (NTYPES, 1)).astype(bfloat16)

        in_maps.append(
            {
                "idxw": idxw,
                "ldst": ldst.astype(bfloat16),
                "nldst": (-ldst).astype(bfloat16),
                "degb": degb,
                "CT": np.ascontiguousarray(
                    Ch[c * SHARD : (c + 1) * SHARD].T
                ).astype(bfloat16),
            }
        )

    # shared (same on all cores) tensors
    J = np.tile(np.arange(128, dtype=np.float32), (128, 1)).astype(bfloat16)
    PIDX = np.arange(128, dtype=np.float32)[:, None]
    embp = np.zeros((NTYPES, DP), np.float32)
    embp[:, :D] = np.asarray(emb, np.float32)
    embp[:, D] = 1.0
    wm = np.zeros((6, DP, DP), np.float32)
    for i, (Ws, Wn, b) in enumerate(Wlist):
        wm[2 * i, :D, :D] = Ws
        wm[2 * i, D, :D] = b
        wm[2 * i, D, D] = 1.0
        wm[2 * i + 1, :D, :D] = Wn
    ident = np.eye(128, dtype=np.float32)
    identb = np.eye(128, dtype=np.float32).astype(bfloat16)
    ones1 = np.ones((1, DP), np.float32)
    shared = {
        "J": J,
        "PIDX": PIDX,
        "embp": embp.astype(bfloat16),
        "wm": wm.astype(bfloat16),
        "ident": ident,
        "identb": identb,
        "ones1": ones1,
    }
    for m in in_maps:
        m.update(shared)
    return in_maps, BE, BO, ecol, ocol, NB, NI


def _build(BE, BO, ecol, ocol, NB, NI):
    import concourse.bass as bass
    import concourse.mybir as mybir
    import concourse.tile as tile
    from concourse import bacc

    dt = mybir.dt
    EQ = mybir.AluOpType.is_equal

    nc = bacc.Bacc(
        "TRN2",
        debug=False,
        num_devices=NCORES,
        dynamic_dma_scratch_size=49152,
        num_swdge_queues=4,
    )

    idxw = nc.dram_tensor("idxw", [128, NI], dt.int16, kind="ExternalInput")
    ldst = nc.dram_tensor("ldst", [128, NB], dt.bfloat16, kind="ExternalInput")
    nldst = nc.dram_tensor("nldst", [128, NB], dt.bfloat16, kind="ExternalInput")
    CTin = nc.dram_tensor("CT", [NTYPES, SHARD], dt.bfloat16, kind="ExternalInput")
    degb = nc.dram_tensor("degb", [NTYPES, SHARD], dt.bfloat16, kind="ExternalInput")
    Jin = nc.dram_tensor("J", [128, 128], dt.bfloat16, kind="ExternalInput")
    PIDXin = nc.dram_tensor("PIDX", [128, 1], dt.float32, kind="ExternalInput")
    embin = nc.dram_tensor("embp", [NTYPES, DP], dt.bfloat16, kind="ExternalInput")
    wmin = nc.dram_tensor("wm", [6, DP, DP], dt.bfloat16, kind="ExternalInput")
    idin = nc.dram_tensor("ident", [128, 128], dt.float32, kind="ExternalInput")
    idbin = nc.dram_tensor("identb", [128, 128], dt.bfloat16, kind="ExternalInput")
    onin = nc.dram_tensor("ones1", [1, DP], dt.float32, kind="ExternalInput")
    y = nc.dram_tensor("y", [SHARD, D], dt.float32, kind="ExternalOutput")

    RG = [list(range(NCORES))]

    with tile.TileContext(nc) as tc:
        with (
            tc.tile_pool(name="dram", bufs=1, space="DRAM") as dram,
            tc.tile_pool(name="persist", bufs=1) as P,
            tc.tile_pool(name="chunks", bufs=1) as CH,
            tc.tile_pool(name="work", bufs=4) as W,
            tc.tile_pool(name="sw", bufs=8) as SW,
            tc.tile_pool(name="gat", bufs=4) as GA,
            tc.tile_pool(name="psum", bufs=5, space="PSUM") as PS,
            tc.tile_pool(name="psb", bufs=2, space="PSUM") as PSB,
        ):
            z0shard = dram.tile([SHARD, 128], dt.bfloat16)
            z1shard = dram.tile([SHARD, 128], dt.bfloat16)
            z0full = dram.tile([NP, 128], dt.bfloat16, addr_space="Shared")
            z1full = dram.tile([NP, 128], dt.bfloat16, addr_space="Shared")

            # ---- preload constants/metadata ----
            idx_sb = P.tile([128, NI], dt.int16)
            nc.sync.dma_start(out=idx_sb[:], in_=idxw[:, :])
            ldstb_sb = P.tile([128, NB], dt.bfloat16)
            nc.sync.dma_start(out=ldstb_sb[:], in_=ldst[:, :])
            nldst_sb = P.tile([128, NB], dt.bfloat16)
            nc.sync.dma_start(out=nldst_sb[:], in_=nldst[:, :])
            degb_sb = P.tile([NTYPES, SHARD], dt.bfloat16)
            nc.sync.dma_start(out=degb_sb[:], in_=degb[:, :])
            J_sb = P.tile([128, 128], dt.bfloat16)
            nc.sync.dma_start(out=J_sb[:], in_=Jin[:, :])
            PIDX_sb = P.tile([128, 1], dt.float32)
            nc.sync.dma_start(out=PIDX_sb[:], in_=PIDXin[:, :])
            emb_sb = P.tile([NTYPES, DP], dt.bfloat16)
            nc.sync.dma_start(out=emb_sb[:], in_=embin[:, :])
            wm_sb = [P.tile([DP, DP], dt.bfloat16, name=f"wm{i}") for i in range(6)]
            for i in range(6):
                nc.sync.dma_start(out=wm_sb[i][:], in_=wmin[i, :, :])
            id_sb = P.tile([128, 128], dt.float32)
            nc.sync.dma_start(out=id_sb[:], in_=idin[:, :])
            idb_sb = P.tile([128, 128], dt.bfloat16)
            nc.sync.dma_start(out=idb_sb[:], in_=idbin[:, :])
            on_sb = P.tile([1, DP], dt.float32)
            nc.sync.dma_start(out=on_sb[:], in_=onin[:, :])

            # persistent transposed chunk arrays [112, 6272] bf16
            feat_all = CH.tile([112, SHARD], dt.bfloat16, name="feat_all")
            aggU_all = CH.tile([112, SHARD], dt.bfloat16, name="aggU_all")
            Z_all = CH.tile([112, SHARD], dt.bfloat16, name="Z_all")
            feat_all2 = CH.tile([112, SHARD], dt.bfloat16, name="feat_all2")
            aggU_all2 = CH.tile([112, SHARD], dt.bfloat16, name="aggU_all2")
            Z_all2 = CH.tile([112, SHARD], dt.bfloat16, name="Z_all2")

            def gslice(g):
                return slice(g * 128, (g + 1) * 128)

            def build_S(col, use_act=False):
                S = SW.tile([128, 128], dt.bfloat16, name="S", tag="S")
                if use_act:
                    # S = relu(1 - |J - ldst|), exact for integer codes
                    St = SW.tile([128, 128], dt.bfloat16, name="St", tag="St")
                    nc.scalar.activation(
                        out=St[:], in_=J_sb[:],
                        func=mybir.ActivationFunctionType.Abs,
                        bias=nldst_sb[:, col : col + 1], scale=1.0,
                    )
                    nc.scalar.activation(
                        out=S[:], in_=St[:],
                        func=mybir.ActivationFunctionType.Relu,
                        bias=1.0, scale=-1.0,
                    )
                else:
                    nc.vector.tensor_tensor(
                        out=S[:],
                        in0=ldstb_sb[:, col : col + 1].to_broadcast([128, 128]),
                        in1=J_sb[:],
                        op=EQ,
                    )
                return S

            def z_pipeline(aggUT_ps, ZT_dst, zshard, g, write_table):
                """aggUT_ps [DP,128] psum f32 -> ZT_dst bf16 slice;
                optionally XBAR + write normal rows to zshard."""
                maxed = W.tile([1, 128], dt.float32, name="maxed", tag="maxed")
                nc.vector.tensor_scalar_max(
                    out=maxed[:], in0=aggUT_ps[D : D + 1, :], scalar1=1.0
                )
                recip = W.tile([1, 128], dt.float32, name="recip", tag="recip")
                nc.vector.reciprocal(out=recip[:], in_=maxed[:])
                bc_ps = PSB.tile([DP, 128], dt.float32, name="bc_ps", tag="bc", bufs=1)
                nc.tensor.matmul(
                    out=bc_ps[:], lhsT=on_sb[:], rhs=recip[:], start=True, stop=True
                )
                bc_sb = W.tile([DP, 128], dt.float32, name="bc_sb", tag="bcs")
                nc.vector.tensor_copy(out=bc_sb[:], in_=bc_ps[:])
                nc.vector.tensor_tensor(
                    out=ZT_dst,
                    in0=aggUT_ps[:DP, :],
                    in1=bc_sb[:],
                    op=mybir.AluOpType.mult,
                )
                if write_table:
                    zn_ps = PSB.tile(
                        [128, 96], dt.bfloat16, name="zn_ps", tag="yt", bufs=1
                    )
                    nc.tensor.transpose(
                        out=zn_ps[:], in_=ZT_dst[0:96, :], identity=idb_sb[:96, :96]
                    )
                    Zn = W.tile([128, 96], dt.bfloat16, name="Zn", tag="Zn")
                    nc.vector.tensor_copy(out=Zn[:], in_=zn_ps[:])
                    nc.sync.dma_start(out=zshard[gslice(g), 0:96], in_=Zn[:])


            OC0 = int(ocol[0])
            necalls = OC0 // 16
            nocalls = (NB - OC0) // 16

            def emit_gathers(zview, phase_tag):
                """Issue all merged dma_gather calls for one SpMM phase,
                E/O interleaved. Returns a block-col -> AP slice fn."""
                etiles = []
                otiles = []
                for k in range(max(necalls, nocalls)):
                    if k < necalls:
                        xt = GA.tile(
                            [128, 16, 128], dt.bfloat16,
                            name=f"XE{phase_tag}", tag="XE",
                        )
                        nc.gpsimd.dma_gather(
                            out_ap=xt[:],
                            in_ap=zview[:, 0:128],
                            idxs_ap=idx_sb[:, k * 128 : (k + 1) * 128],
                            num_idxs=2048,
                            num_idxs_reg=2048,
                            elem_size=128,
                            elem_step=256,
                            single_packet=False,
                            queue_num=(2 * k) % 4,
                        )
                        etiles.append(xt)
                    if k < nocalls:
                        ko = OC0 // 16 + k
                        xt = GA.tile(
                            [128, 16, 128], dt.bfloat16,
                            name=f"XO{phase_tag}", tag="XO",
                        )
                        nc.gpsimd.dma_gather(
                            out_ap=xt[:],
                            in_ap=zview[:, 128:256],
                            idxs_ap=idx_sb[:, ko * 128 : (ko + 1) * 128],
                            num_idxs=2048,
                            num_idxs_reg=2048,
                            elem_size=128,
                            elem_step=256,
                            single_packet=False,
                            queue_num=(2 * k + 1) % 4,
                        )
                        otiles.append(xt)

                def xslice(col):
                    t = etiles[col // 16] if col < OC0 else otiles[(col - OC0) // 16]
                    return t[:, col % 16, 0:D]

                return xslice

            # ================= P0: feat_0, C, aggU_0, Z_0 =================
            for g in range(GP):
                OHT = W.tile([NTYPES, 128], dt.bfloat16, name="OHT", tag="OHT")
                nc.vector.tensor_scalar(
                    out=OHT[:], in0=degb_sb[:, gslice(g)],
                    scalar1=PIDX_sb[:NTYPES, :], scalar2=None, op0=EQ,
                )
                f0_ps = PS.tile([DP, 128], dt.float32, name="f0_ps", tag="mm")
                nc.tensor.matmul(
                    out=f0_ps[:], lhsT=emb_sb[:], rhs=OHT[:], start=True, stop=True
                )
                nc.vector.tensor_copy(
                    out=feat_all[:DP, gslice(g)], in_=f0_ps[:]
                )

                ct_sb = W.tile([NTYPES, 128], dt.bfloat16, name="ct_sb", tag="cts")
                nc.sync.dma_start(out=ct_sb[:], in_=CTin[:, gslice(g)])
                a0_ps = PS.tile([DP, 128], dt.float32, name="a0_ps", tag="mm")
                nc.tensor.matmul(
                    out=a0_ps[:], lhsT=emb_sb[:], rhs=ct_sb[:], start=True, stop=True
                )
                nc.vector.tensor_copy(out=aggU_all[:DP, gslice(g)], in_=a0_ps[:])
                z_pipeline(a0_ps, Z_all[:DP, gslice(g)], z0shard, g, True)

            nc.gpsimd.collective_compute(
                "AllGather",
                mybir.AluOpType.bypass,
                replica_groups=RG,
                ins=[z0shard[:, :].opt()],
                outs=[z0full[:, :].opt()],
            )

            # ============== SpMM phase template =================
            def spmm_phase(
                zfull, feat_src, aggU_src, Z_src, feat_dst, aggU_dst, Z_dst,
                wS, wN, zshard_out, phase_tag, write_table,
            ):
                zview = zfull[:, :].rearrange("(n two) d -> n (two d)", two=2)
                for g in range(GP):
                    # feat_next = feat @ Ws' + Z @ Wn' (no gather dependency;
                    # overlaps the preceding AllGather)
                    fn_ps = PS.tile([DP, 128], dt.float32, name="fn_ps", tag="mm")
                    nc.tensor.matmul(
                        out=fn_ps[:], lhsT=wS[:], rhs=feat_src[:DP, gslice(g)],
                        start=True, stop=False,
                    )
                    nc.tensor.matmul(
                        out=fn_ps[:], lhsT=wN[:], rhs=Z_src[:DP, gslice(g)],
                        start=False, stop=True,
                    )
                    nc.vector.tensor_copy(out=feat_dst[:DP, gslice(g)], in_=fn_ps[:])
                xslice = emit_gathers(zview, phase_tag)
                for g in range(GP):
                    be, bo = int(BE[g]), int(BO[g])
                    m_ps = PS.tile([DP, 128], dt.float32, name="m_ps", tag="mm")
                    for b in range(be + bo):
                        col = int(ecol[g]) + b if b < be else int(ocol[g]) + b - be
                        S = build_S(col, use_act=(b % 2 == 1))
                        xsl = xslice(col)
                        nc.tensor.matmul(
                            out=m_ps[:D, :],
                            lhsT=xsl,
                            rhs=S[:],
                            start=(b == 0),
                            stop=(b == be + bo - 1),
                        )
                    m_sb = W.tile([D, 128], dt.bfloat16, name="m_sb", tag="msb")
                    nc.vector.tensor_copy(out=m_sb[:], in_=m_ps[:D, :])

                    # aggU_next = aggU @ Ws' + M @ Wn'
                    an_ps = PS.tile([DP, 128], dt.float32, name="an_ps", tag="mm")
                    nc.tensor.matmul(
                        out=an_ps[:], lhsT=wS[:], rhs=aggU_src[:DP, gslice(g)],
                        start=True, stop=False,
                    )
                    nc.tensor.matmul(
                        out=an_ps[:], lhsT=wN[:D, :], rhs=m_sb[:], start=False,
                        stop=True,
                    )
                    if aggU_dst is not None:
                        nc.vector.tensor_copy(
                            out=aggU_dst[:DP, gslice(g)], in_=an_ps[:]
                        )
                    z_pipeline(
                        an_ps, Z_dst[:DP, gslice(g)], zshard_out, g, write_table
                    )

            # ========== P1: M_0, aggU_1, feat_1, Z_1 ==========
            spmm_phase(
                z0full, feat_all, aggU_all, Z_all,
                feat_all2, aggU_all2, Z_all2,
                wm_sb[0], wm_sb[1], z1shard, "p1", True,
            )
            nc.gpsimd.collective_compute(
                "AllGather",
                mybir.AluOpType.bypass,
                replica_groups=RG,
                ins=[z1shard[:, :].opt()],
                outs=[z1full[:, :].opt()],
            )

            # ========== P2: M_1, aggU_2, Z_2, feat_2, feat_3, output ==========
            feat2_all = CH.tile([112, SHARD], dt.bfloat16, name="feat2_all")
            zview1 = z1full[:, :].rearrange("(n two) d -> n (two d)", two=2)
            for g in range(GP):
                f2_ps = PS.tile([DP, 128], dt.float32, name="f2_ps", tag="mm")
                nc.tensor.matmul(
                    out=f2_ps[:], lhsT=wm_sb[2][:], rhs=feat_all2[:DP, gslice(g)],
                    start=True, stop=False,
                )
                nc.tensor.matmul(
                    out=f2_ps[:], lhsT=wm_sb[3][:], rhs=Z_all2[:DP, gslice(g)],
                    start=False, stop=True,
                )
                nc.vector.tensor_copy(out=feat2_all[:DP, gslice(g)], in_=f2_ps[:])
            xslice2 = emit_gathers(zview1, "p2")
            for g in range(GP):
                be, bo = int(BE[g]), int(BO[g])
                m_ps = PS.tile([DP, 128], dt.float32, name="m_ps2", tag="mm")
                for b in range(be + bo):
                    col = int(ecol[g]) + b if b < be else int(ocol[g]) + b - be
                    S = build_S(col, use_act=(b % 2 == 1))
                    xsl = xslice2(col)
                    nc.tensor.matmul(
                        out=m_ps[:D, :], lhsT=xsl, rhs=S[:],
                        start=(b == 0), stop=(b == be + bo - 1),
                    )
                m_sb = W.tile([D, 128], dt.bfloat16, name="m_sb2", tag="msb")
                nc.vector.tensor_copy(out=m_sb[:], in_=m_ps[:D, :])

                a2_ps = PS.tile([DP, 128], dt.float32, name="a2_ps", tag="mm")
                nc.tensor.matmul(
                    out=a2_ps[:], lhsT=wm_sb[2][:], rhs=aggU_all2[:DP, gslice(g)],
                    start=True, stop=False,
                )
                nc.tensor.matmul(
                    out=a2_ps[:], lhsT=wm_sb[3][:D, :], rhs=m_sb[:], start=False,
                    stop=True,
                )
                z2t = W.tile([DP, 128], dt.bfloat16, name="z2t", tag="z2t")
                z_pipeline(a2_ps, z2t[:], None, g, False)

                f3_ps = PS.tile([DP, 128], dt.float32, name="f3_ps", tag="mm")
                nc.tensor.matmul(
                    out=f3_ps[:], lhsT=wm_sb[4][:], rhs=feat2_all[:DP, gslice(g)],
                    start=True, stop=False,
                )
                nc.tensor.matmul(
                    out=f3_ps[:], lhsT=wm_sb[5][:], rhs=z2t[:], start=False, stop=True
                )
                f3_sb = W.tile([D, 128], dt.float32, name="f3_sb", tag="f3s")
                nc.vector.tensor_copy(out=f3_sb[:], in_=f3_ps[:D, :])
                yt_ps = PSB.tile([128, D], dt.float32, name="yt_ps", tag="yt", bufs=1)
                nc.tensor.transpose(
                    out=yt_ps[:], in_=f3_sb[:], identity=id_sb[:D, :D]
                )
                y_sb = W.tile([128, D], dt.float32, name="y_sb", tag="ys")
                nc.vector.tensor_copy(out=y_sb[:], in_=yt_ps[:])
                nc.sync.dma_start(out=y[gslice(g), :], in_=y_sb[:])

    nc.compile()
    return nc


def kernel(degree, edge_src, edge_dst, emb, Ws0, Wn0, b0, Ws1, Wn1, b1, Ws2, Wn2, b2,
           _trace=False):
    from concourse import bass_utils

    Wlist = [
        (np.asarray(Ws0, np.float32), np.asarray(Wn0, np.float32), np.asarray(b0, np.float32)),
        (np.asarray(Ws1, np.float32), np.asarray(Wn1, np.float32), np.asarray(b1, np.float32)),
        (np.asarray(Ws2, np.float32), np.asarray(Wn2, np.float32), np.asarray(b2, np.float32)),
    ]
    in_maps, BE, BO, ecol, ocol, NB, NI = _prep(degree, edge_src, edge_dst, emb, Wlist)
    nc = _build(BE, BO, ecol, ocol, NB, NI)
    res = bass_utils.run_bass_kernel_spmd(
        nc, in_maps=in_maps, core_ids=list(range(NCORES)), trace=_trace
    )
    out = np.concatenate([res.results[c]["y"] for c in range(NCORES)], axis=0)[:N]
    kernel.last_exec_time_ns = res.exec_time_ns
    return out.astype(np.float32)
